# revision 1
# baseline (speedup 1.0000x reference)
# Mistral sliding-window attention (B=1, S=2048, H=4096, 32 q heads / 8 kv
# heads, window 4096 -> plain causal at this S) on 8 Trainium2 NeuronCores.
#
# Sharding: tensor-parallel over heads. Core c owns q heads 4c..4c+3 and kv
# head c. hidden_states is replicated (transposed on host to [H, S] so the
# contraction dim is the partition dim). Each core computes its attention
# output slice attn.T [512, S]; per-head AllGathers assemble the full
# [4096, S] while later heads still compute, and each core accumulates a
# 512-column slice of o_proj head-by-head; the host concatenates the 8
# column slices into the full output.
#
# All big matmuls run as float32r (fp32 storage, full-rate PE) with the
# moving dim = 512. Scores are computed transposed (S.T[kv, q]) so that the
# P@V contraction needs no transposes of the probability tiles; softmax
# denominators come from an all-ones stationary matmul accumulated alongside
# P@V, and the causal mask is a host-precomputed staircase slice multiplied
# in after exp. Attention runs two-pass per (head, q-chunk) — all score
# matmuls + exps first, then the PV/sum matmuls — so the PE never stalls on
# the ACT engine mid-chain.

from contextlib import ExitStack

import numpy as np

import concourse.bacc as bacc
import concourse.bass as bass
import concourse.mybir as mybir
import concourse.tile as tile
from concourse.bass_utils import run_bass_kernel_spmd
from concourse.masks import make_identity

HIDDEN = 4096
NH = 32
NKV = 8
HD = 128
THETA = 10000.0
S = 2048
NCORES = 8

QH = NH // NCORES          # 4 q heads per core
DQ = QH * HD               # 512 (per-core q/attn width)
DOUT = DQ + 2 * HD         # 768 = q heads + k + v projection width
MT = DOUT // 128           # 6 projection m-tiles (0..3 q, 4 k, 5 v)
KT = HIDDEN // 128         # 32 contraction tiles
KG = 4                     # x-load group: k-tiles per DMA
TCH = 512                  # token chunk (matmul moving dim)
NTCH = S // TCH            # 4
KVT = S // 128             # 16 kv tiles
SCALE = 1.0 / float(np.sqrt(HD))

F32 = mybir.dt.float32
F32R = mybir.dt.float32r
EXP = mybir.ActivationFunctionType.Exp


def _rope(nc, pool, src, dst, cs, sn):
    """dst = src*cos + rotate_half(src)*sin, in [d, tok] layout.

    src/dst are [128, n]; cs/sn are [64, n] (the two 64-row halves share
    frequencies). rotate_half: rows 0:64 get -src[64:128], rows 64:128 get
    src[0:64].
    """
    top, bot = src[0:64, :], src[64:128, :]
    ta = pool.tile([64, TCH], F32, name="rope_a")
    tb = pool.tile([64, TCH], F32, name="rope_b")
    nc.vector.tensor_mul(ta, top, cs)
    nc.vector.tensor_mul(tb, bot, sn)
    nc.vector.tensor_sub(dst[0:64, :], ta, tb)
    nc.vector.tensor_mul(ta, bot, cs)
    nc.vector.tensor_mul(tb, top, sn)
    nc.vector.tensor_add(dst[64:128, :], ta, tb)


def build_kernel_body(ctx: ExitStack, tc: tile.TileContext, outs, ins):
    nc = tc.nc
    xT, wqkv, ow, cos_t, sin_t, stair = (
        ins["xT"], ins["wqkv"], ins["ow"], ins["cos_t"], ins["sin_t"], ins["stair"],
    )
    out = outs["out"]

    # per-head bounce + gather buffers so each head's AllGather can fire as
    # soon as that head's attention is done (overlaps comm with compute)
    attn_loc = [nc.dram_tensor(f"attn_loc{h}", [HD, S], F32).ap()
                for h in range(QH)]
    attn_gat = [nc.dram_tensor(f"attn_gat{h}", [NCORES * HD, S], F32,
                               addr_space="Shared").ap()
                for h in range(QH)]

    singles = ctx.enter_context(tc.tile_pool(name="singles", bufs=1))
    stair_sb = singles.tile([128, 896], F32)
    nc.sync.dma_start(out=stair_sb, in_=stair)
    ones_sb = singles.tile([128, 128], F32R)

    # persistent projection outputs, [d, tok] layout
    qT = singles.tile([128, QH, S], F32R)    # q head h -> qT[:, h, :]
    kT = singles.tile([128, S], F32R)
    V = singles.tile([128, KVT, HD], F32R)   # V[:, j, :] = [tok 128, d 128]

    # ---- phase 1: QKV projection + RoPE --------------------------------
    with (
        tc.tile_pool(name="wq", bufs=1) as wp,
        tc.tile_pool(name="xt", bufs=3) as xp,
        tc.tile_pool(name="rope", bufs=2) as rp,
        tc.tile_pool(name="p1ps", bufs=1, space="PSUM") as pp1,
    ):
        cos_sb = wp.tile([64, S], F32)
        sin_sb = wp.tile([64, S], F32)
        vT = wp.tile([128, S], F32)
        ident_sb = wp.tile([128, 128], F32)
        ones_f = wp.tile([128, 128], F32)
        nc.vector.memset(ones_f, 1.0)
        nc.vector.tensor_copy(ones_sb, ones_f)
        make_identity(nc, ident_sb)

        wq3 = wqkv.rearrange("(k p) d -> p k d", p=128)
        x3 = xT.rearrange("(k p) s -> p k s", p=128)
        # x chunk (t=0, kg=0) first so the first matmul starts almost
        # immediately; weight k-tiles follow in per-tile DMAs
        w_sb = [wp.tile([128, DOUT], F32R, name=f"w{k}", tag=f"w{k}")
                for k in range(KT)]
        xg0 = xp.tile([128, KG, TCH], F32R, name="xg")
        nc.sync.dma_start(out=xg0, in_=x3[:, 0:KG, 0:TCH])
        for k in range(KT):
            nc.sync.dma_start(out=w_sb[k], in_=wq3[:, k, :])
        nc.sync.dma_start(out=cos_sb, in_=cos_t)
        nc.sync.dma_start(out=sin_sb, in_=sin_t)
        for t in range(NTCH):
            ps = [pp1.tile([128, TCH], F32, name=f"p1_{m}", tag=f"p1_{m}")
                  for m in range(MT)]
            for kg in range(KT // KG):
                if t == 0 and kg == 0:
                    xg = xg0
                else:
                    xg = xp.tile([128, KG, TCH], F32R, name="xg")
                    nc.sync.dma_start(
                        out=xg,
                        in_=x3[:, kg * KG:(kg + 1) * KG, t * TCH:(t + 1) * TCH])
                for ki in range(KG):
                    k = kg * KG + ki
                    for m in range(MT):
                        nc.tensor.matmul(
                            ps[m],
                            lhsT=w_sb[k][:, m * 128:(m + 1) * 128],
                            rhs=xg[:, ki, :],
                            start=(k == 0), stop=(k == KT - 1),
                        )
            cs = cos_sb[:, t * TCH:(t + 1) * TCH]
            sn = sin_sb[:, t * TCH:(t + 1) * TCH]
            for h in range(QH):
                _rope(nc, rp, ps[h], qT[:, h, t * TCH:(t + 1) * TCH], cs, sn)
            _rope(nc, rp, ps[QH], kT[:, t * TCH:(t + 1) * TCH], cs, sn)
            nc.scalar.copy(out=vT[:, t * TCH:(t + 1) * TCH], in_=ps[QH + 1])
            # V = vT.T for this chunk's kv tiles (PE transpose [d,tok]->[tok,d])
            for j in range(4 * t, 4 * t + 4):
                pv = pp1.tile([128, 128], F32, name="pvt", tag="pvt")
                nc.tensor.transpose(pv, vT[:, j * 128:(j + 1) * 128], ident_sb)
                nc.scalar.copy(out=V[:, j, :], in_=pv)

    # ---- phases 2+3: attention heads with per-head AllGather; o_proj for
    # head 0 interleaved into attention head 3, rest at the tail ---------
    with (
        tc.tile_pool(name="pt", bufs=8) as ptp,
        tc.tile_pool(name="ao", bufs=2) as aop,
        tc.tile_pool(name="ow", bufs=16) as owp,
        tc.tile_pool(name="at", bufs=2) as atp,
        tc.tile_pool(name="acc", bufs=1) as accp,
        tc.tile_pool(name="p2sc", bufs=2, space="PSUM") as pp2,
        tc.tile_pool(name="p2acc", bufs=1, space="PSUM") as pa2,
        tc.tile_pool(name="p3ps", bufs=1, space="PSUM") as pp3,
    ):
        # o_proj output accumulator: acc[:, b, :] = out rows b*128:(b+1)*128
        acc = accp.tile([128, S // 128, TCH], F32)
        ow3 = ow.rearrange("(k p) d -> p k d", p=128)
        ag3 = [attn_gat[h].rearrange("(r p) s -> p r s", p=128)
               for h in range(QH)]

        def attention_chunk(h, c):
            """One (head, q-chunk): scores+exp in j-pair waves, then PV."""
            jmax = 4 * c + 3
            po = pa2.tile([128, TCH], F32, name="po", tag="po")
            psum_s = pa2.tile([128, TCH], F32, name="ps", tag="ps")
            qslice = qT[:, h, c * TCH:(c + 1) * TCH]
            npair = (jmax + 1) // 2
            PW = 4  # j-pairs per pass-A/B wave (bounds live pt tiles)
            for p0 in range(0, npair, PW):
                p1 = min(p0 + PW, npair)
                pts = []
                for p in range(p0, p1):
                    # two score matmuls into one 2-bank psum tile, one exp
                    sc = pp2.tile([128, 2, TCH], F32, name="sc", tag="sc")
                    for i in range(2):
                        j = 2 * p + i
                        nc.tensor.matmul(
                            sc[:, i, :], lhsT=kT[:, j * 128:(j + 1) * 128],
                            rhs=qslice, start=True, stop=True)
                    pt = ptp.tile([128, 2, TCH], F32R, name="pt", tag="pt")
                    nc.scalar.activation(pt, sc, EXP, scale=SCALE)
                    for i in range(2):
                        j = 2 * p + i
                        rdiag = j - 4 * c
                        if rdiag >= 0:  # tile touches the causal diagonal
                            off = 384 - rdiag * 128
                            nc.vector.tensor_mul(
                                pt[:, i, :], pt[:, i, :],
                                stair_sb[:, off:off + TCH])
                    pts.append(pt)
                for idx, p in enumerate(range(p0, p1)):
                    for i in range(2):
                        j = 2 * p + i
                        nc.tensor.matmul(po, lhsT=V[:, j, :],
                                         rhs=pts[idx][:, i, :],
                                         start=(j == 0), stop=(j == jmax))
                        nc.tensor.matmul(psum_s, lhsT=ones_sb,
                                         rhs=pts[idx][:, i, :],
                                         start=(j == 0), stop=(j == jmax))
            rec = aop.tile([128, TCH], F32, name="rec")
            nc.vector.reciprocal(rec, psum_s)
            ao = aop.tile([128, TCH], F32, name="ao")
            nc.vector.tensor_mul(ao, po, rec)
            nc.sync.dma_start(
                out=attn_loc[h][:, c * TCH:(c + 1) * TCH], in_=ao)

        def allgather_head(h):
            nc.gpsimd.collective_compute(
                "AllGather",
                mybir.AluOpType.bypass,
                ins=[attn_loc[h][:, :]],
                outs=[attn_gat[h][:, :]],
                replica_groups=[list(range(NCORES))],
            )

        def oproj_load_weights(h):
            ows = []
            for r in range(NCORES):
                owk = owp.tile([128, DQ], F32R, name="owk", tag="owk")
                nc.sync.dma_start(out=owk, in_=ow3[:, r * QH + h, :])
                ows.append(owk)
            return ows

        def oproj_chunk(h, g, ows):
            """acc[:, 4g:4g+4, :] += sum_r at(r, h) @ ow(r, h) for 512 toks."""
            at = atp.tile([128, NCORES, TCH], F32R, name="at", tag="at")
            nc.sync.dma_start(
                out=at, in_=ag3[h][:, :, g * TCH:(g + 1) * TCH].bitcast(F32R))
            for mp in range(2):
                pcs = [pp3.tile([128, TCH], F32, name=f"pc{i}", tag=f"pc{i}")
                       for i in range(2)]
                for r in range(NCORES):
                    for i, mi in enumerate((2 * mp, 2 * mp + 1)):
                        nc.tensor.matmul(
                            pcs[i],
                            lhsT=at[:, r, mi * 128:(mi + 1) * 128],
                            rhs=ows[r],
                            start=(r == 0), stop=(r == NCORES - 1),
                        )
                for i, mi in enumerate((2 * mp, 2 * mp + 1)):
                    b = g * 4 + mi
                    if h == 0:
                        nc.scalar.copy(out=acc[:, b, :], in_=pcs[i])
                    else:
                        nc.vector.tensor_add(acc[:, b, :], acc[:, b, :],
                                             pcs[i])

        for h in range(3):
            for c in range(NTCH):
                attention_chunk(h, c)
            allgather_head(h)
        # head 3 attention interleaved with o_proj of the gathered head 0
        ows0 = oproj_load_weights(0)
        for c in range(NTCH):
            attention_chunk(3, c)
            oproj_chunk(0, c, ows0)
        allgather_head(3)
        for h in range(1, QH):
            ows = oproj_load_weights(h)
            for g in range(S // TCH):
                oproj_chunk(h, g, ows)

        nc.sync.dma_start(out=out.rearrange("(b p) d -> p b d", p=128), in_=acc)


_NC_CACHE = None


def build_program():
    global _NC_CACHE
    if _NC_CACHE is not None:
        return _NC_CACHE
    nc = bacc.Bacc("TRN2", target_bir_lowering=False, debug=False,
                   num_devices=NCORES)
    ins = {
        "xT": nc.dram_tensor("xT", [HIDDEN, S], F32R, kind="ExternalInput").ap(),
        "wqkv": nc.dram_tensor("wqkv", [HIDDEN, DOUT], F32R,
                               kind="ExternalInput").ap(),
        "ow": nc.dram_tensor("ow", [HIDDEN, DQ], F32R, kind="ExternalInput").ap(),
        "cos_t": nc.dram_tensor("cos_t", [64, S], F32, kind="ExternalInput").ap(),
        "sin_t": nc.dram_tensor("sin_t", [64, S], F32, kind="ExternalInput").ap(),
        "stair": nc.dram_tensor("stair", [128, 896], F32,
                                kind="ExternalInput").ap(),
    }
    outs = {"out": nc.dram_tensor("out", [S, DQ], F32, kind="ExternalOutput").ap()}
    with tile.TileContext(nc) as tc:
        with ExitStack() as ctx:
            build_kernel_body(ctx, tc, outs, ins)
    nc.compile()
    _NC_CACHE = nc
    return nc


def make_in_maps(hidden_states, position_ids, q_w, k_w, v_w, o_w):
    x = np.asarray(hidden_states, dtype=np.float32).reshape(S, HIDDEN)
    xT = np.ascontiguousarray(x.T)
    pos = np.asarray(position_ids).reshape(S).astype(np.float64)
    inv = 1.0 / (THETA ** (np.arange(0, HD, 2, dtype=np.float64) / HD))
    fr = inv[:, None] * pos[None, :]                       # [64, S]
    cos_t = np.cos(fr).astype(np.float32)
    sin_t = np.sin(fr).astype(np.float32)
    u = np.arange(896, dtype=np.int64)[None, :]
    kvi = np.arange(128, dtype=np.int64)[:, None]
    stair = ((u - kvi) >= 384).astype(np.float32)          # [128, 896]

    q_w = np.asarray(q_w, dtype=np.float32)
    k_w = np.asarray(k_w, dtype=np.float32)
    v_w = np.asarray(v_w, dtype=np.float32)
    o_w = np.asarray(o_w, dtype=np.float32)

    in_maps = []
    for c in range(NCORES):
        wqkv = np.ascontiguousarray(np.concatenate(
            [q_w[:, c * DQ:(c + 1) * DQ],
             k_w[:, c * HD:(c + 1) * HD],
             v_w[:, c * HD:(c + 1) * HD]], axis=1))
        owc = np.ascontiguousarray(o_w[:, c * DQ:(c + 1) * DQ])
        in_maps.append({"xT": xT, "wqkv": wqkv, "ow": owc,
                        "cos_t": cos_t, "sin_t": sin_t, "stair": stair})
    return in_maps


def run(inputs: dict, trace: bool = False):
    """Run on the 8 NeuronCores; returns (full_output, BassKernelResults)."""
    nc = build_program()
    in_maps = make_in_maps(**inputs)
    res = run_bass_kernel_spmd(nc, in_maps, core_ids=list(range(NCORES)),
                               trace=trace)
    full = np.concatenate([res.results[c]["out"] for c in range(NCORES)], axis=1)
    return full.reshape(1, S, HIDDEN), res


def kernel(**inputs) -> np.ndarray:
    out, _ = run(inputs)
    return out



# revision 3
# speedup vs baseline: 1.4609x; 1.4609x over previous
# Mistral sliding-window attention (B=1, S=2048, H=4096, 32 q heads / 8 kv
# heads, window 4096 -> plain causal at this S) on 8 Trainium2 NeuronCores.
#
# Sharding: tensor-parallel over heads with NO on-device collectives. Core c
# owns q heads 4c..4c+3 and kv head c. hidden_states is replicated
# (transposed on host to [H, S]). Each core computes attention for its 4
# heads and then a PARTIAL o_proj over the FULL 4096 output columns using
# only its own 512 attention dims; the host sums the 8 partial outputs.
# This removes the AllGather serialization entirely.
#
# All matmul operands are bf16 (psum accumulation stays fp32): same PE
# stream rate as fp32r but half the DMA/SBUF traffic and half the
# LDWEIGHTS time. Scores are computed transposed (S.T[kv, q]); softmax
# denominators come from an all-ones stationary matmul accumulated
# alongside P@V; the causal mask is a host-precomputed staircase slice
# multiplied in after exp. The attention units (head, chunk) are
# software-pipelined: scores+exp of unit u+1 are emitted before P@V of
# unit u, and the o_proj of chunk c runs right after its 4 heads finish,
# draining psum->sbuf (bf16) -> DRAM.

from contextlib import ExitStack

import numpy as np
import ml_dtypes

import concourse.bacc as bacc
import concourse.bass as bass
import concourse.mybir as mybir
import concourse.tile as tile
from concourse.bass_utils import run_bass_kernel_spmd
from concourse.masks import make_identity

HIDDEN = 4096
NH = 32
NKV = 8
HD = 128
THETA = 10000.0
S = 2048
NCORES = 8

QH = NH // NCORES          # 4 q heads per core
DQ = QH * HD               # 512 (per-core attn width)
DOUT = DQ + 2 * HD         # 768 = q heads + k + v projection width
MT = DOUT // 128           # 6 projection m-tiles (0..3 q, 4 k, 5 v)
KT = HIDDEN // 128         # 32 contraction tiles
KG = 4                     # x-load group: k-tiles per DMA
TCH = 512                  # token chunk (matmul moving dim)
NTCH = S // TCH            # 4
KVT = S // 128             # 16 kv tiles
HG = HIDDEN // TCH         # 8 o_proj output column groups
SCALE = 1.0 / float(np.sqrt(HD))

F32 = mybir.dt.float32
BF16 = mybir.dt.bfloat16
EXP = mybir.ActivationFunctionType.Exp


def _rope(nc, pool, src, dst, cs, sn):
    """dst = src*cos + rotate_half(src)*sin, in [d, tok] layout.

    src is [128, n] (psum f32); dst is [128, n] (sbuf, any dtype);
    cs/sn are [64, n] f32 (the two 64-row halves share frequencies).
    """
    top, bot = src[0:64, :], src[64:128, :]
    ta = pool.tile([64, TCH], F32, name="rope_a")
    tb = pool.tile([64, TCH], F32, name="rope_b")
    nc.vector.tensor_mul(ta, top, cs)
    nc.vector.tensor_mul(tb, bot, sn)
    nc.vector.tensor_sub(dst[0:64, :], ta, tb)
    nc.vector.tensor_mul(ta, bot, cs)
    nc.vector.tensor_mul(tb, top, sn)
    nc.vector.tensor_add(dst[64:128, :], ta, tb)


def build_kernel_body(ctx: ExitStack, tc: tile.TileContext, outs, ins):
    nc = tc.nc
    xT, wqkv, ow, cos_t, sin_t, stair = (
        ins["xT"], ins["wqkv"], ins["ow"], ins["cos_t"], ins["sin_t"], ins["stair"],
    )
    out = outs["out"]

    singles = ctx.enter_context(tc.tile_pool(name="singles", bufs=1))
    stair_sb = singles.tile([128, 896], BF16)
    ones_sb = singles.tile([128, 128], BF16)
    ow_sb = singles.tile([128, QH, HIDDEN], BF16)   # [d, head, hid]

    # persistent projection outputs, [d, tok] layout
    qT = singles.tile([128, QH, S], BF16)    # q head h -> qT[:, h, :]
    kT = singles.tile([128, S], BF16)
    V = singles.tile([128, KVT, HD], BF16)   # V[:, j, :] = [tok 128, d 128]

    # ---- phase 1: QKV projection + RoPE --------------------------------
    with (
        tc.tile_pool(name="wq", bufs=1) as wp,
        tc.tile_pool(name="xt", bufs=3) as xp,
        tc.tile_pool(name="rope", bufs=2) as rp,
        tc.tile_pool(name="p1ps", bufs=1, space="PSUM") as pp1,
    ):
        cos_sb = wp.tile([64, S], F32)
        sin_sb = wp.tile([64, S], F32)
        vT = wp.tile([128, S], BF16)
        ident_sb = wp.tile([128, 128], BF16)
        nc.vector.memset(ones_sb, 1.0)
        make_identity(nc, ident_sb)

        wq3 = wqkv.rearrange("(k p) d -> p k d", p=128)
        x3 = xT.rearrange("(k p) s -> p k s", p=128)
        # x chunk (t=0, kg=0) first so the first matmul starts almost
        # immediately; weight k-tiles follow in per-tile DMAs
        w_sb = [wp.tile([128, DOUT], BF16, name=f"w{k}", tag=f"w{k}")
                for k in range(KT)]
        xg0 = xp.tile([128, KG, TCH], BF16, name="xg")
        nc.sync.dma_start(out=xg0, in_=x3[:, 0:KG, 0:TCH])
        for k in range(KT):
            nc.sync.dma_start(out=w_sb[k], in_=wq3[:, k, :])
        nc.sync.dma_start(out=cos_sb, in_=cos_t)
        nc.sync.dma_start(out=sin_sb, in_=sin_t)
        nc.sync.dma_start(out=stair_sb, in_=stair)
        nc.sync.dma_start(out=ow_sb, in_=ow)
        for t in range(NTCH):
            ps = [pp1.tile([128, TCH], F32, name=f"p1_{m}", tag=f"p1_{m}")
                  for m in range(MT)]
            for kg in range(KT // KG):
                if t == 0 and kg == 0:
                    xg = xg0
                else:
                    xg = xp.tile([128, KG, TCH], BF16, name="xg")
                    nc.sync.dma_start(
                        out=xg,
                        in_=x3[:, kg * KG:(kg + 1) * KG, t * TCH:(t + 1) * TCH])
                for ki in range(KG):
                    k = kg * KG + ki
                    for m in range(MT):
                        nc.tensor.matmul(
                            ps[m],
                            lhsT=w_sb[k][:, m * 128:(m + 1) * 128],
                            rhs=xg[:, ki, :],
                            start=(k == 0), stop=(k == KT - 1),
                        )
            cs = cos_sb[:, t * TCH:(t + 1) * TCH]
            sn = sin_sb[:, t * TCH:(t + 1) * TCH]
            for h in range(QH):
                _rope(nc, rp, ps[h], qT[:, h, t * TCH:(t + 1) * TCH], cs, sn)
            _rope(nc, rp, ps[QH], kT[:, t * TCH:(t + 1) * TCH], cs, sn)
            nc.scalar.copy(out=vT[:, t * TCH:(t + 1) * TCH], in_=ps[QH + 1])
            # V = vT.T for this chunk's kv tiles (PE transpose [d,tok]->[tok,d])
            for j in range(4 * t, 4 * t + 4):
                pv = pp1.tile([128, 128], BF16, name="pvt", tag="pvt")
                nc.tensor.transpose(pv, vT[:, j * 128:(j + 1) * 128], ident_sb)
                nc.scalar.copy(out=V[:, j, :], in_=pv)

    # ---- phase 2: attention + partial o_proj, software-pipelined -------
    with (
        tc.tile_pool(name="pt", bufs=34) as ptp,
        tc.tile_pool(name="ao", bufs=8) as aop,
        tc.tile_pool(name="rc", bufs=2) as rcp,
        tc.tile_pool(name="ob", bufs=4) as obp,
        tc.tile_pool(name="p2sc", bufs=2, space="PSUM") as pp2,
        tc.tile_pool(name="p2po", bufs=2, space="PSUM") as pop,
        tc.tile_pool(name="p2dn", bufs=2, space="PSUM") as dnp,
        tc.tile_pool(name="p2op", bufs=2, space="PSUM") as opp,
    ):
        def attn_S(h, c):
            """Scores + exp + causal stair for one (head, q-chunk)."""
            pts = []
            qslice = qT[:, h, c * TCH:(c + 1) * TCH]
            for j in range(4 * c + 4):
                sc = pp2.tile([128, TCH], F32, name="sc", tag="sc")
                nc.tensor.matmul(sc, lhsT=kT[:, j * 128:(j + 1) * 128],
                                 rhs=qslice, start=True, stop=True)
                pt = ptp.tile([128, TCH], BF16, name="pt", tag="pt")
                nc.scalar.activation(pt, sc, EXP, scale=SCALE)
                rdiag = j - 4 * c
                if rdiag >= 0:  # tile touches the causal diagonal
                    off = 384 - rdiag * 128
                    nc.vector.tensor_mul(pt, pt, stair_sb[:, off:off + TCH])
                pts.append(pt)
            return pts

        def attn_PV(h, c, pts):
            """P@V + denominator + normalize for one (head, q-chunk)."""
            jmax = 4 * c + 3
            po = pop.tile([128, TCH], F32, name="po", tag="po")
            den = dnp.tile([128, TCH], F32, name="den", tag="den")
            for j, pt in enumerate(pts):
                nc.tensor.matmul(po, lhsT=V[:, j, :], rhs=pt,
                                 start=(j == 0), stop=(j == jmax))
                nc.tensor.matmul(den, lhsT=ones_sb, rhs=pt,
                                 start=(j == 0), stop=(j == jmax))
            rec = rcp.tile([128, TCH], F32, name="rec")
            nc.vector.reciprocal_approx_fast(rec, den)
            ao = aop.tile([128, TCH], BF16, name="ao")
            nc.vector.tensor_mul(ao, po, rec)
            return ao

        def oproj(c, aos):
            """Partial o_proj for token chunk c: out[tok, :] over all 4096
            columns, contracting this core's 4 heads (512 attn dims)."""
            for ts in range(TCH // 128):
                for hg in range(HG):
                    op = opp.tile([128, TCH], F32, name="op", tag="op")
                    for h in range(QH):
                        nc.tensor.matmul(
                            op,
                            lhsT=aos[h][:, ts * 128:(ts + 1) * 128],
                            rhs=ow_sb[:, h, hg * TCH:(hg + 1) * TCH],
                            start=(h == 0), stop=(h == QH - 1),
                        )
                    ob = obp.tile([128, TCH], BF16, name="ob")
                    nc.scalar.copy(out=ob, in_=op)
                    r0 = c * TCH + ts * 128
                    nc.sync.dma_start(
                        out=out[r0:r0 + 128, hg * TCH:(hg + 1) * TCH], in_=ob)

        units = [(c, h) for c in range(NTCH) for h in range(QH)]
        pts_cur = attn_S(units[0][1], units[0][0])
        aos = []
        for idx, (c, h) in enumerate(units):
            if idx + 1 < len(units):
                c2, h2 = units[idx + 1]
                pts_nxt = attn_S(h2, c2)
            else:
                pts_nxt = None
            aos.append(attn_PV(h, c, pts_cur))
            pts_cur = pts_nxt
            if h == QH - 1:
                oproj(c, aos)
                aos = []


_NC_CACHE = None


def build_program():
    global _NC_CACHE
    if _NC_CACHE is not None:
        return _NC_CACHE
    nc = bacc.Bacc("TRN2", target_bir_lowering=False, debug=False,
                   num_devices=NCORES)
    ins = {
        "xT": nc.dram_tensor("xT", [HIDDEN, S], BF16, kind="ExternalInput").ap(),
        "wqkv": nc.dram_tensor("wqkv", [HIDDEN, DOUT], BF16,
                               kind="ExternalInput").ap(),
        "ow": nc.dram_tensor("ow", [128, QH, HIDDEN], BF16,
                             kind="ExternalInput").ap(),
        "cos_t": nc.dram_tensor("cos_t", [64, S], F32, kind="ExternalInput").ap(),
        "sin_t": nc.dram_tensor("sin_t", [64, S], F32, kind="ExternalInput").ap(),
        "stair": nc.dram_tensor("stair", [128, 896], BF16,
                                kind="ExternalInput").ap(),
    }
    outs = {"out": nc.dram_tensor("out", [S, HIDDEN], BF16,
                                  kind="ExternalOutput").ap()}
    with tile.TileContext(nc) as tc:
        with ExitStack() as ctx:
            build_kernel_body(ctx, tc, outs, ins)
    nc.compile()
    _NC_CACHE = nc
    return nc


def make_in_maps(hidden_states, position_ids, q_w, k_w, v_w, o_w):
    bf16 = ml_dtypes.bfloat16
    x = np.asarray(hidden_states, dtype=np.float32).reshape(S, HIDDEN)
    xT = np.ascontiguousarray(x.T).astype(bf16)
    pos = np.asarray(position_ids).reshape(S).astype(np.float64)
    inv = 1.0 / (THETA ** (np.arange(0, HD, 2, dtype=np.float64) / HD))
    fr = inv[:, None] * pos[None, :]                       # [64, S]
    cos_t = np.cos(fr).astype(np.float32)
    sin_t = np.sin(fr).astype(np.float32)
    u = np.arange(896, dtype=np.int64)[None, :]
    kvi = np.arange(128, dtype=np.int64)[:, None]
    stair = ((u - kvi) >= 384).astype(bf16)                # [128, 896]

    q_w = np.asarray(q_w, dtype=np.float32)
    k_w = np.asarray(k_w, dtype=np.float32)
    v_w = np.asarray(v_w, dtype=np.float32)
    o_w = np.asarray(o_w, dtype=np.float32)

    in_maps = []
    for c in range(NCORES):
        wqkv = np.ascontiguousarray(np.concatenate(
            [q_w[:, c * DQ:(c + 1) * DQ],
             k_w[:, c * HD:(c + 1) * HD],
             v_w[:, c * HD:(c + 1) * HD]], axis=1)).astype(bf16)
        # o_w rows for this core's 512 attn dims -> [d 128, head 4, hid 4096]
        owc = np.ascontiguousarray(
            o_w[c * DQ:(c + 1) * DQ, :].reshape(QH, HD, HIDDEN)
            .transpose(1, 0, 2)).astype(bf16)
        in_maps.append({"xT": xT, "wqkv": wqkv, "ow": owc,
                        "cos_t": cos_t, "sin_t": sin_t, "stair": stair})
    return in_maps


def run(inputs: dict, trace: bool = False):
    """Run on the 8 NeuronCores; returns (full_output, BassKernelResults)."""
    nc = build_program()
    in_maps = make_in_maps(**inputs)
    res = run_bass_kernel_spmd(nc, in_maps, core_ids=list(range(NCORES)),
                               trace=trace)
    acc = np.zeros((S, HIDDEN), dtype=np.float32)
    for c in range(NCORES):
        acc += np.asarray(res.results[c]["out"], dtype=np.float32)
    return acc.reshape(1, S, HIDDEN), res


def kernel(**inputs) -> np.ndarray:
    out, _ = run(inputs)
    return out


# revision 6
# speedup vs baseline: 1.6165x; 1.1065x over previous
# Mistral sliding-window attention (B=1, S=2048, H=4096, 32 q heads / 8 kv
# heads, window 4096 -> plain causal at this S) on 8 Trainium2 NeuronCores.
#
# Sharding: tensor-parallel over heads with NO on-device collectives. Core c
# owns q heads 4c..4c+3 and kv head c. hidden_states is replicated
# (transposed on host to [H, S]). Each core computes attention for its 4
# heads and then a PARTIAL o_proj over the FULL 4096 output columns using
# only its own 512 attention dims; the host sums the 8 partial outputs.
# This removes the AllGather serialization entirely.
#
# All matmul operands are bf16 (psum accumulation stays fp32): same PE
# stream rate as fp32r but half the DMA/SBUF traffic and half the
# LDWEIGHTS time. Scores are computed transposed (S.T[kv, q]); softmax
# denominators come from an all-ones stationary matmul accumulated
# alongside P@V; the causal mask is a host-precomputed staircase slice
# multiplied in after exp. The attention units (head, chunk) are
# software-pipelined: scores+exp of unit u+1 are emitted before P@V of
# unit u, and the o_proj of chunk c runs right after its 4 heads finish,
# draining psum->sbuf (bf16) -> DRAM.

from contextlib import ExitStack

import numpy as np
import ml_dtypes

import concourse.bacc as bacc
import concourse.bass as bass
import concourse.mybir as mybir
import concourse.tile as tile
from concourse.bass_utils import run_bass_kernel_spmd
from concourse.masks import make_identity

HIDDEN = 4096
NH = 32
NKV = 8
HD = 128
THETA = 10000.0
S = 2048
NCORES = 8

QH = NH // NCORES          # 4 q heads per core
DQ = QH * HD               # 512 (per-core attn width)
DOUT = DQ + 2 * HD         # 768 = q heads + k + v projection width
MT = DOUT // 128           # 6 projection m-tiles (0..3 q, 4 k, 5 v)
KT = HIDDEN // 128         # 32 contraction tiles
KG = 4                     # x-load group: k-tiles per DMA
TCH = 512                  # token chunk (matmul moving dim)
NTCH = S // TCH            # 4
KVT = S // 128             # 16 kv tiles
HG = HIDDEN // TCH         # 8 o_proj output column groups
SCALE = 1.0 / float(np.sqrt(HD))

F32 = mybir.dt.float32
BF16 = mybir.dt.bfloat16
EXP = mybir.ActivationFunctionType.Exp


def _rope(nc, pool, src, dst, cs, sn):
    """dst = src*cos + rotate_half(src)*sin, in [d, tok] layout.

    src is [128, n] (psum f32); dst is [128, n] (sbuf, any dtype);
    cs/sn are [64, n] f32 (the two 64-row halves share frequencies).
    """
    top, bot = src[0:64, :], src[64:128, :]
    ta = pool.tile([64, TCH], F32, name="rope_a")
    tb = pool.tile([64, TCH], F32, name="rope_b")
    nc.vector.tensor_mul(ta, top, cs)
    nc.vector.tensor_mul(tb, bot, sn)
    nc.vector.tensor_sub(dst[0:64, :], ta, tb)
    nc.vector.tensor_mul(ta, bot, cs)
    nc.vector.tensor_mul(tb, top, sn)
    nc.vector.tensor_add(dst[64:128, :], ta, tb)


def build_kernel_body(ctx: ExitStack, tc: tile.TileContext, outs, ins):
    nc = tc.nc
    xT, wqkv, ow, cos_t, sin_t, stair = (
        ins["xT"], ins["wqkv"], ins["ow"], ins["cos_t"], ins["sin_t"], ins["stair"],
    )
    out = outs["out"]

    singles = ctx.enter_context(tc.tile_pool(name="singles", bufs=1))
    stair_sb = singles.tile([128, 896], BF16)
    ones_sb = singles.tile([128, 128], BF16)
    ow_sb = singles.tile([128, QH, HIDDEN], BF16)   # [d, head, hid]

    # persistent projection outputs, [d, tok] layout
    qT = singles.tile([128, QH, S], BF16)    # q head h -> qT[:, h, :]
    kT = singles.tile([128, S], BF16)
    V = singles.tile([128, KVT, HD], BF16)   # V[:, j, :] = [tok 128, d 128]

    # ---- phase 1: QKV projection + RoPE --------------------------------
    with (
        tc.tile_pool(name="wq", bufs=1) as wp,
        tc.tile_pool(name="xt", bufs=3) as xp,
        tc.tile_pool(name="rope", bufs=2) as rp,
        tc.tile_pool(name="p1ps", bufs=1, space="PSUM") as pp1,
    ):
        cos_sb = wp.tile([64, S], F32)
        sin_sb = wp.tile([64, S], F32)
        vT = wp.tile([128, S], BF16)
        ident_sb = wp.tile([128, 128], BF16)
        nc.vector.memset(ones_sb, 1.0)
        make_identity(nc, ident_sb)

        wq3 = wqkv.rearrange("(k p) d -> p k d", p=128)
        x3 = xT.rearrange("(k p) s -> p k s", p=128)
        # x chunk (t=0, kg=0) first so the first matmul starts almost
        # immediately; weight k-tiles follow in per-tile DMAs
        w_sb = [wp.tile([128, DOUT], BF16, name=f"w{k}", tag=f"w{k}")
                for k in range(KT)]
        # interleave t=0 x-group loads with the weight k-tiles so neither
        # starves the first matmul chain; cos/sin next (needed by rope t=0);
        # stair/ow_sb (needed much later) go after the whole t-loop
        xg_t0 = [xp.tile([128, KG, TCH], BF16, name="xg", tag=f"xg{g % 3}")
                 for g in range(KT // KG)]
        nc.sync.dma_start(out=xg_t0[0], in_=x3[:, 0:KG, 0:TCH])
        for kg in range(KT // KG):
            for k in range(kg * KG, (kg + 1) * KG):
                nc.sync.dma_start(out=w_sb[k], in_=wq3[:, k, :])
            if kg + 1 < KT // KG:
                nc.sync.dma_start(
                    out=xg_t0[kg + 1],
                    in_=x3[:, (kg + 1) * KG:(kg + 2) * KG, 0:TCH])
        nc.sync.dma_start(out=cos_sb, in_=cos_t)
        nc.sync.dma_start(out=sin_sb, in_=sin_t)
        for t in range(NTCH):
            ps = [pp1.tile([128, TCH], F32, name=f"p1_{m}", tag=f"p1_{m}")
                  for m in range(MT)]
            for kg in range(KT // KG):
                if t == 0:
                    xg = xg_t0[kg]
                else:
                    xg = xp.tile([128, KG, TCH], BF16, name="xg",
                                 tag=f"xg{kg % 3}")
                    nc.sync.dma_start(
                        out=xg,
                        in_=x3[:, kg * KG:(kg + 1) * KG, t * TCH:(t + 1) * TCH])
                for ki in range(KG):
                    k = kg * KG + ki
                    for m in range(MT):
                        nc.tensor.matmul(
                            ps[m],
                            lhsT=w_sb[k][:, m * 128:(m + 1) * 128],
                            rhs=xg[:, ki, :],
                            start=(k == 0), stop=(k == KT - 1),
                        )
            cs = cos_sb[:, t * TCH:(t + 1) * TCH]
            sn = sin_sb[:, t * TCH:(t + 1) * TCH]
            for h in range(QH):
                _rope(nc, rp, ps[h], qT[:, h, t * TCH:(t + 1) * TCH], cs, sn)
            _rope(nc, rp, ps[QH], kT[:, t * TCH:(t + 1) * TCH], cs, sn)
            nc.scalar.copy(out=vT[:, t * TCH:(t + 1) * TCH], in_=ps[QH + 1])
            # V = vT.T for this chunk's kv tiles (PE transpose [d,tok]->[tok,d])
            for j in range(4 * t, 4 * t + 4):
                pv = pp1.tile([128, 128], BF16, name="pvt", tag="pvt")
                nc.tensor.transpose(pv, vT[:, j * 128:(j + 1) * 128], ident_sb)
                nc.scalar.copy(out=V[:, j, :], in_=pv)
            if t == 0:
                nc.sync.dma_start(out=stair_sb, in_=stair)
                nc.sync.dma_start(out=ow_sb, in_=ow)

    # ---- phase 2: attention + partial o_proj, software-pipelined -------
    with (
        tc.tile_pool(name="pt", bufs=34) as ptp,
        tc.tile_pool(name="ao", bufs=8) as aop,
        tc.tile_pool(name="rc", bufs=2) as rcp,
        tc.tile_pool(name="ob", bufs=4) as obp,
        tc.tile_pool(name="p2sc", bufs=2, space="PSUM") as pp2,
        tc.tile_pool(name="p2po", bufs=2, space="PSUM") as pop,
        tc.tile_pool(name="p2dn", bufs=2, space="PSUM") as dnp,
        tc.tile_pool(name="p2op", bufs=2, space="PSUM") as opp,
    ):
        def attn_S(h, c):
            """Scores + exp + causal stair for one (head, q-chunk)."""
            pts = []
            qslice = qT[:, h, c * TCH:(c + 1) * TCH]
            for j in range(4 * c + 4):
                sc = pp2.tile([128, TCH], F32, name="sc", tag="sc")
                nc.tensor.matmul(sc, lhsT=kT[:, j * 128:(j + 1) * 128],
                                 rhs=qslice, start=True, stop=True)
                pt = ptp.tile([128, TCH], BF16, name="pt", tag="pt")
                nc.scalar.activation(pt, sc, EXP, scale=SCALE)
                rdiag = j - 4 * c
                if rdiag >= 0:  # tile touches the causal diagonal
                    off = 384 - rdiag * 128
                    nc.vector.tensor_mul(pt, pt, stair_sb[:, off:off + TCH])
                pts.append(pt)
            return pts

        def attn_PV(h, c, pts):
            """P@V + denominator + normalize for one (head, q-chunk)."""
            jmax = 4 * c + 3
            po = pop.tile([128, TCH], F32, name="po", tag="po")
            den = dnp.tile([128, TCH], F32, name="den", tag="den")
            for j, pt in enumerate(pts):
                nc.tensor.matmul(po, lhsT=V[:, j, :], rhs=pt,
                                 start=(j == 0), stop=(j == jmax))
                nc.tensor.matmul(den, lhsT=ones_sb, rhs=pt,
                                 start=(j == 0), stop=(j == jmax))
            rec = rcp.tile([128, TCH], F32, name="rec")
            nc.vector.reciprocal_approx_fast(rec, den)
            ao = aop.tile([128, TCH], BF16, name="ao")
            nc.vector.tensor_mul(ao, po, rec)
            return ao

        def oproj(c, aos):
            """Partial o_proj for token chunk c: out[tok, :] over all 4096
            columns, contracting this core's 4 heads (512 attn dims)."""
            for ts in range(TCH // 128):
                for hg in range(HG):
                    op = opp.tile([128, TCH], F32, name="op", tag="op")
                    for h in range(QH):
                        nc.tensor.matmul(
                            op,
                            lhsT=aos[h][:, ts * 128:(ts + 1) * 128],
                            rhs=ow_sb[:, h, hg * TCH:(hg + 1) * TCH],
                            start=(h == 0), stop=(h == QH - 1),
                        )
                    ob = obp.tile([128, TCH], BF16, name="ob")
                    nc.vector.tensor_copy(ob, op)
                    r0 = c * TCH + ts * 128
                    nc.sync.dma_start(
                        out=out[r0:r0 + 128, hg * TCH:(hg + 1) * TCH], in_=ob)

        units = [(c, h) for c in range(NTCH) for h in range(QH)]
        pts_cur = attn_S(units[0][1], units[0][0])
        aos = []
        for idx, (c, h) in enumerate(units):
            if idx + 1 < len(units):
                c2, h2 = units[idx + 1]
                pts_nxt = attn_S(h2, c2)
            else:
                pts_nxt = None
            aos.append(attn_PV(h, c, pts_cur))
            pts_cur = pts_nxt
            if h == QH - 1:
                oproj(c, aos)
                aos = []


_NC_CACHE = None


def build_program():
    global _NC_CACHE
    if _NC_CACHE is not None:
        return _NC_CACHE
    nc = bacc.Bacc("TRN2", target_bir_lowering=False, debug=False,
                   num_devices=NCORES)
    ins = {
        "xT": nc.dram_tensor("xT", [HIDDEN, S], BF16, kind="ExternalInput").ap(),
        "wqkv": nc.dram_tensor("wqkv", [HIDDEN, DOUT], BF16,
                               kind="ExternalInput").ap(),
        "ow": nc.dram_tensor("ow", [128, QH, HIDDEN], BF16,
                             kind="ExternalInput").ap(),
        "cos_t": nc.dram_tensor("cos_t", [64, S], F32, kind="ExternalInput").ap(),
        "sin_t": nc.dram_tensor("sin_t", [64, S], F32, kind="ExternalInput").ap(),
        "stair": nc.dram_tensor("stair", [128, 896], BF16,
                                kind="ExternalInput").ap(),
    }
    outs = {"out": nc.dram_tensor("out", [S, HIDDEN], BF16,
                                  kind="ExternalOutput").ap()}
    with tile.TileContext(nc) as tc:
        with ExitStack() as ctx:
            build_kernel_body(ctx, tc, outs, ins)
    nc.compile()
    _NC_CACHE = nc
    return nc


def make_in_maps(hidden_states, position_ids, q_w, k_w, v_w, o_w):
    bf16 = ml_dtypes.bfloat16
    x = np.asarray(hidden_states, dtype=np.float32).reshape(S, HIDDEN)
    xT = np.ascontiguousarray(x.T).astype(bf16)
    pos = np.asarray(position_ids).reshape(S).astype(np.float64)
    inv = 1.0 / (THETA ** (np.arange(0, HD, 2, dtype=np.float64) / HD))
    fr = inv[:, None] * pos[None, :]                       # [64, S]
    cos_t = np.cos(fr).astype(np.float32)
    sin_t = np.sin(fr).astype(np.float32)
    u = np.arange(896, dtype=np.int64)[None, :]
    kvi = np.arange(128, dtype=np.int64)[:, None]
    stair = ((u - kvi) >= 384).astype(bf16)                # [128, 896]

    q_w = np.asarray(q_w, dtype=np.float32)
    k_w = np.asarray(k_w, dtype=np.float32)
    v_w = np.asarray(v_w, dtype=np.float32)
    o_w = np.asarray(o_w, dtype=np.float32)

    in_maps = []
    for c in range(NCORES):
        wqkv = np.ascontiguousarray(np.concatenate(
            [q_w[:, c * DQ:(c + 1) * DQ],
             k_w[:, c * HD:(c + 1) * HD],
             v_w[:, c * HD:(c + 1) * HD]], axis=1)).astype(bf16)
        # o_w rows for this core's 512 attn dims -> [d 128, head 4, hid 4096]
        owc = np.ascontiguousarray(
            o_w[c * DQ:(c + 1) * DQ, :].reshape(QH, HD, HIDDEN)
            .transpose(1, 0, 2)).astype(bf16)
        in_maps.append({"xT": xT, "wqkv": wqkv, "ow": owc,
                        "cos_t": cos_t, "sin_t": sin_t, "stair": stair})
    return in_maps


def run(inputs: dict, trace: bool = False):
    """Run on the 8 NeuronCores; returns (full_output, BassKernelResults)."""
    nc = build_program()
    in_maps = make_in_maps(**inputs)
    res = run_bass_kernel_spmd(nc, in_maps, core_ids=list(range(NCORES)),
                               trace=trace)
    acc = np.zeros((S, HIDDEN), dtype=np.float32)
    for c in range(NCORES):
        acc += np.asarray(res.results[c]["out"], dtype=np.float32)
    return acc.reshape(1, S, HIDDEN), res


def kernel(**inputs) -> np.ndarray:
    out, _ = run(inputs)
    return out


# revision 18
# speedup vs baseline: 1.6255x; 1.0056x over previous
# Mistral sliding-window attention (B=1, S=2048, H=4096, 32 q heads / 8 kv
# heads, window 4096 -> plain causal at this S) on 8 Trainium2 NeuronCores.
#
# Sharding: tensor-parallel over heads with NO on-device collectives. Core c
# owns q heads 4c..4c+3 and kv head c. hidden_states is replicated
# (transposed on host to [H, S]). Each core computes attention for its 4
# heads and then a PARTIAL o_proj over the FULL 4096 output columns using
# only its own 512 attention dims; the host sums the 8 partial outputs.
# This removes the AllGather serialization entirely.
#
# All matmul operands are bf16 (psum accumulation stays fp32): same PE
# stream rate as fp32r but half the DMA/SBUF traffic and half the
# LDWEIGHTS time. Scores are computed transposed (S.T[kv, q]); softmax
# denominators come from an all-ones stationary matmul accumulated
# alongside P@V; the causal mask is a host-precomputed staircase slice
# multiplied in after exp. The attention units (head, chunk) are
# software-pipelined: scores+exp of unit u+1 are emitted before P@V of
# unit u, and the o_proj of chunk c runs right after its 4 heads finish,
# draining psum->sbuf (bf16) -> DRAM.

from contextlib import ExitStack

import numpy as np
import ml_dtypes

import concourse.bacc as bacc
import concourse.bass as bass
import concourse.mybir as mybir
import concourse.tile as tile
from concourse.bass_utils import run_bass_kernel_spmd
from concourse.masks import make_identity

HIDDEN = 4096
NH = 32
NKV = 8
HD = 128
THETA = 10000.0
S = 2048
NCORES = 8

QH = NH // NCORES          # 4 q heads per core
DQ = QH * HD               # 512 (per-core attn width)
DOUT = DQ + 2 * HD         # 768 = q heads + k + v projection width
MT = DOUT // 128           # 6 projection m-tiles (0..3 q, 4 k, 5 v)
KT = HIDDEN // 128         # 32 contraction tiles
KG = 4                     # x-load group: k-tiles per DMA
TCH = 512                  # token chunk (matmul moving dim)
NTCH = S // TCH            # 4
KVT = S // 128             # 16 kv tiles
HG = HIDDEN // TCH         # 8 o_proj output column groups
SCALE = 1.0 / float(np.sqrt(HD))

F32 = mybir.dt.float32
BF16 = mybir.dt.bfloat16
FP8 = mybir.dt.float8e4
EXP = mybir.ActivationFunctionType.Exp
WSCALE = 64.0   # qkv weights pre-scaled by this for fp8 (avoids subnormals)


def _rope(nc, pool, src, dst, cs, sn):
    """dst = src*cos + rotate_half(src)*sin, in [d, tok] layout.

    src is [128, n] (psum f32); dst is [128, n] (sbuf, any dtype);
    cs/sn are [64, n] f32 (the two 64-row halves share frequencies).
    """
    top, bot = src[0:64, :], src[64:128, :]
    ta = pool.tile([64, TCH], F32, name="rope_a")
    tb = pool.tile([64, TCH], F32, name="rope_b")
    nc.vector.tensor_mul(ta, top, cs)
    nc.vector.tensor_mul(tb, bot, sn)
    nc.vector.tensor_sub(dst[0:64, :], ta, tb)
    nc.vector.tensor_mul(ta, bot, cs)
    nc.vector.tensor_mul(tb, top, sn)
    nc.vector.tensor_add(dst[64:128, :], ta, tb)


def build_kernel_body(ctx: ExitStack, tc: tile.TileContext, outs, ins):
    nc = tc.nc
    xT, wqkv, ow, cos_t, sin_t, stair = (
        ins["xT"], ins["wqkv"], ins["ow"], ins["cos_t"], ins["sin_t"], ins["stair"],
    )
    out = outs["out"]

    singles = ctx.enter_context(tc.tile_pool(name="singles", bufs=1))
    stair_sb = singles.tile([128, 896], BF16)
    ones_sb = singles.tile([128, 128], BF16)
    ow_sb = singles.tile([128, QH, HIDDEN], BF16)   # [d, head, hid]

    # persistent projection outputs, [d, tok] layout; one tile per token
    # chunk so attention units only depend on the chunks they read
    qTc = [singles.tile([128, QH, TCH], BF16, name=f"qT{t}")
           for t in range(NTCH)]
    kTc = [singles.tile([128, TCH], BF16, name=f"kT{t}")
           for t in range(NTCH)]
    Vc = [singles.tile([128, 4, HD], BF16, name=f"V{t}")
          for t in range(NTCH)]              # Vc[t][:, j%4, :] = [tok, d]

    # ---- phase 1: QKV projection + RoPE --------------------------------
    with (
        tc.tile_pool(name="wq", bufs=1) as wp,
        tc.tile_pool(name="xt", bufs=3) as xp,
        tc.tile_pool(name="rope", bufs=2) as rp,
        tc.tile_pool(name="p1ps", bufs=1, space="PSUM") as pp1,
    ):
        cos_sb = wp.tile([64, S], F32)
        sin_sb = wp.tile([64, S], F32)
        vT = wp.tile([128, S], BF16)
        ident_sb = wp.tile([128, 128], BF16)
        nc.vector.memset(ones_sb, 1.0)
        make_identity(nc, ident_sb)

        wq3 = wqkv.rearrange("(a p) d -> p a d", p=128)   # [128, KT, DOUT]
        x3 = xT.rearrange("(k p) s -> p k s", p=128)
        # interleave t=0 x-group loads with the weight k-tiles so neither
        # starves the first matmul chain; cos/sin next (needed by rope t=0);
        # stair/ow_sb (needed much later) go after chunk 0.
        w_sb = [wp.tile([128, DOUT], BF16, name=f"w{k}", tag=f"w{k}")
                for k in range(KT)]
        xg_t0 = [xp.tile([128, KG, TCH], BF16, name="xg", tag=f"xg{g % 3}")
                 for g in range(KT // KG)]
        nc.sync.dma_start(out=xg_t0[0], in_=x3[:, 0:KG, 0:TCH])
        for kg in range(KT // KG):
            for k in range(kg * KG, (kg + 1) * KG):
                nc.sync.dma_start(out=w_sb[k], in_=wq3[:, k, :])
            if kg + 1 < KT // KG:
                nc.sync.dma_start(
                    out=xg_t0[kg + 1],
                    in_=x3[:, (kg + 1) * KG:(kg + 2) * KG, 0:TCH])
        nc.sync.dma_start(out=cos_sb, in_=cos_t)
        nc.sync.dma_start(out=sin_sb, in_=sin_t)
        for t in range(NTCH):
            ps = [pp1.tile([128, TCH], F32, name=f"p1_{m}", tag=f"p1_{m}")
                  for m in range(MT)]
            for kg in range(KT // KG):
                if t == 0:
                    xg = xg_t0[kg]
                else:
                    xg = xp.tile([128, KG, TCH], BF16, name="xg",
                                 tag=f"xg{kg % 3}")
                    nc.sync.dma_start(
                        out=xg,
                        in_=x3[:, kg * KG:(kg + 1) * KG, t * TCH:(t + 1) * TCH])
                for ki in range(KG):
                    k = kg * KG + ki
                    for m in range(MT):
                        nc.tensor.matmul(
                            ps[m],
                            lhsT=w_sb[k][:, m * 128:(m + 1) * 128],
                            rhs=xg[:, ki, :],
                            start=(k == 0), stop=(k == KT - 1),
                        )
            cs = cos_sb[:, t * TCH:(t + 1) * TCH]
            sn = sin_sb[:, t * TCH:(t + 1) * TCH]
            for h in range(QH):
                _rope(nc, rp, ps[h], qTc[t][:, h, :], cs, sn)
            _rope(nc, rp, ps[QH], kTc[t], cs, sn)
            nc.scalar.copy(out=vT[:, t * TCH:(t + 1) * TCH], in_=ps[QH + 1])
            # V = vT.T for this chunk's kv tiles (PE transpose [d,tok]->[tok,d])
            for j in range(4):
                jj = 4 * t + j
                pv = pp1.tile([128, 128], BF16, name="pvt", tag="pvt")
                nc.tensor.transpose(pv, vT[:, jj * 128:(jj + 1) * 128],
                                    ident_sb)
                nc.scalar.copy(out=Vc[t][:, j, :], in_=pv)
            if t == 0:
                nc.sync.dma_start(out=stair_sb, in_=stair)
                nc.sync.dma_start(out=ow_sb, in_=ow)

    # ---- phase 2: attention + partial o_proj, software-pipelined -------
    with (
        tc.tile_pool(name="pt", bufs=34) as ptp,
        tc.tile_pool(name="ao", bufs=8) as aop,
        tc.tile_pool(name="rc", bufs=2) as rcp,
        tc.tile_pool(name="ob", bufs=4) as obp,
        tc.tile_pool(name="p2sc", bufs=2, space="PSUM") as pp2,
        tc.tile_pool(name="p2po", bufs=2, space="PSUM") as pop,
        tc.tile_pool(name="p2dn", bufs=2, space="PSUM") as dnp,
        tc.tile_pool(name="p2op", bufs=2, space="PSUM") as opp,
    ):
        def attn_S(h, c):
            """Scores + exp + causal stair for one (head, q-chunk)."""
            pts = []
            qslice = qTc[c][:, h, :]
            for j in range(4 * c + 4):
                sc = pp2.tile([128, TCH], F32, name="sc", tag="sc")
                nc.tensor.matmul(sc,
                                 lhsT=kTc[j // 4][:, (j % 4) * 128:
                                                  (j % 4 + 1) * 128],
                                 rhs=qslice, start=True, stop=True)
                pt = ptp.tile([128, TCH], BF16, name="pt", tag="pt")
                nc.scalar.activation(pt, sc, EXP, scale=SCALE)
                rdiag = j - 4 * c
                if rdiag >= 0:  # tile touches the causal diagonal
                    off = 384 - rdiag * 128
                    nc.vector.tensor_mul(pt, pt, stair_sb[:, off:off + TCH])
                pts.append(pt)
            return pts

        def attn_PV(h, c, pts):
            """P@V + denominator + normalize for one (head, q-chunk)."""
            jmax = 4 * c + 3
            po = pop.tile([128, TCH], F32, name="po", tag="po")
            den = dnp.tile([128, TCH], F32, name="den", tag="den")
            for j, pt in enumerate(pts):
                nc.tensor.matmul(po, lhsT=Vc[j // 4][:, j % 4, :], rhs=pt,
                                 start=(j == 0), stop=(j == jmax))
                nc.tensor.matmul(den, lhsT=ones_sb, rhs=pt,
                                 start=(j == 0), stop=(j == jmax))
            rec = rcp.tile([128, TCH], F32, name="rec")
            nc.vector.reciprocal_approx_fast(rec, den)
            ao = aop.tile([128, TCH], BF16, name="ao")
            nc.vector.tensor_mul(ao, po, rec)
            return ao

        def oproj(c, aos):
            """Partial o_proj for token chunk c: out[tok, :] over all 4096
            columns, contracting this core's 4 heads (512 attn dims)."""
            for ts in range(TCH // 128):
                for hg in range(HG):
                    op = opp.tile([128, TCH], F32, name="op", tag="op")
                    for h in range(QH):
                        nc.tensor.matmul(
                            op,
                            lhsT=aos[h][:, ts * 128:(ts + 1) * 128],
                            rhs=ow_sb[:, h, hg * TCH:(hg + 1) * TCH],
                            start=(h == 0), stop=(h == QH - 1),
                        )
                    ob = obp.tile([128, TCH], BF16, name="ob")
                    nc.vector.tensor_copy(ob, op)
                    r0 = c * TCH + ts * 128
                    nc.sync.dma_start(
                        out=out[r0:r0 + 128, hg * TCH:(hg + 1) * TCH], in_=ob)

        units = [(c, h) for c in range(NTCH) for h in range(QH)]
        pts_cur = attn_S(units[0][1], units[0][0])
        aos = []
        for idx, (c, h) in enumerate(units):
            if idx + 1 < len(units):
                c2, h2 = units[idx + 1]
                pts_nxt = attn_S(h2, c2)
            else:
                pts_nxt = None
            aos.append(attn_PV(h, c, pts_cur))
            pts_cur = pts_nxt
            if h == QH - 1:
                oproj(c, aos)
                aos = []


_NC_CACHE = None


def build_program():
    global _NC_CACHE
    if _NC_CACHE is not None:
        return _NC_CACHE
    nc = bacc.Bacc("TRN2", target_bir_lowering=False, debug=False,
                   num_devices=NCORES)
    ins = {
        "xT": nc.dram_tensor("xT", [HIDDEN, S], BF16, kind="ExternalInput").ap(),
        "wqkv": nc.dram_tensor("wqkv", [HIDDEN, DOUT], BF16,
                               kind="ExternalInput").ap(),
        "ow": nc.dram_tensor("ow", [128, QH, HIDDEN], BF16,
                             kind="ExternalInput").ap(),
        "cos_t": nc.dram_tensor("cos_t", [64, S], F32, kind="ExternalInput").ap(),
        "sin_t": nc.dram_tensor("sin_t", [64, S], F32, kind="ExternalInput").ap(),
        "stair": nc.dram_tensor("stair", [128, 896], BF16,
                                kind="ExternalInput").ap(),
    }
    outs = {"out": nc.dram_tensor("out", [S, HIDDEN], BF16,
                                  kind="ExternalOutput").ap()}
    with tile.TileContext(nc) as tc:
        with ExitStack() as ctx:
            build_kernel_body(ctx, tc, outs, ins)
    nc.compile()
    _NC_CACHE = nc
    return nc


def make_in_maps(hidden_states, position_ids, q_w, k_w, v_w, o_w):
    bf16 = ml_dtypes.bfloat16
    x = np.asarray(hidden_states, dtype=np.float32).reshape(S, HIDDEN)
    xT = np.ascontiguousarray(x.T).astype(bf16)
    pos = np.asarray(position_ids).reshape(S).astype(np.float64)
    inv = 1.0 / (THETA ** (np.arange(0, HD, 2, dtype=np.float64) / HD))
    fr = inv[:, None] * pos[None, :]                       # [64, S]
    cos_t = np.cos(fr).astype(np.float32)
    sin_t = np.sin(fr).astype(np.float32)
    u = np.arange(896, dtype=np.int64)[None, :]
    kvi = np.arange(128, dtype=np.int64)[:, None]
    stair = ((u - kvi) >= 384).astype(bf16)                # [128, 896]

    q_w = np.asarray(q_w, dtype=np.float32)
    k_w = np.asarray(k_w, dtype=np.float32)
    v_w = np.asarray(v_w, dtype=np.float32)
    o_w = np.asarray(o_w, dtype=np.float32)

    in_maps = []
    for c in range(NCORES):
        wqkv = np.ascontiguousarray(np.concatenate(
            [q_w[:, c * DQ:(c + 1) * DQ],
             k_w[:, c * HD:(c + 1) * HD],
             v_w[:, c * HD:(c + 1) * HD]], axis=1)).astype(bf16)
        # o_w rows for this core's 512 attn dims -> [d 128, head 4, hid 4096]
        owc = np.ascontiguousarray(
            o_w[c * DQ:(c + 1) * DQ, :].reshape(QH, HD, HIDDEN)
            .transpose(1, 0, 2)).astype(bf16)
        in_maps.append({"xT": xT, "wqkv": wqkv, "ow": owc,
                        "cos_t": cos_t, "sin_t": sin_t, "stair": stair})
    return in_maps


def run(inputs: dict, trace: bool = False):
    """Run on the 8 NeuronCores; returns (full_output, BassKernelResults)."""
    nc = build_program()
    in_maps = make_in_maps(**inputs)
    res = run_bass_kernel_spmd(nc, in_maps, core_ids=list(range(NCORES)),
                               trace=trace)
    acc = np.zeros((S, HIDDEN), dtype=np.float32)
    for c in range(NCORES):
        acc += np.asarray(res.results[c]["out"], dtype=np.float32)
    return acc.reshape(1, S, HIDDEN), res


def kernel(**inputs) -> np.ndarray:
    out, _ = run(inputs)
    return out
